# revision 4
# baseline (speedup 1.0000x reference)
"""KV-cache ring-buffer update + rolling re-linearization on 8 trn2 cores.

Problem semantics (nn_KVCache): scatter k/v into the ring buffer at pos,
then roll to logical order. For the given pos/max_pos this reduces to
contiguous row copies per batch:
  out[:, 0:7168]    = cache[:, 512:7680]
  out[:, 7168:8192] = new[:, 0:1024]
Sharding: pure batch-parallel (B=8 -> 1 batch per core), no communication.
Each core runs DRAM->DRAM HWDGE DMA copies on both hardware DGE queues.

DMA schedule per core (established empirically by tracing):
- HWDGE assigns ceil(c/16) descriptors per SDMA engine, filling engines from
  slot 0: a 15-desc instruction puts nothing on engine 15 (the slow one).
- 64-KiB descriptors (the HW max: 16-bit byte-size ISA field).
- ~34 instructions per queue (dispatch costs ~850 ns each on the issuing
  engine, so instruction count is kept moderate).
- Engine 15 biased to ~0.75-0.8 of the other engines' bytes (it is
  intermittently ~15-20% slower; underloading it is cheap insurance).
"""

import numpy as np

B, S_NEW, H, D = 8, 1024, 16, 128
MAX_SIZE = 8192
HD = H * D
N_CORES = 8

D16 = 32768            # 16-desc chunk descriptor size (elems), 65536 B (HW max)
D15 = 32760            # 15-desc chunk descriptor size (elems), 65520 B
T16 = 16 * D16         # 1048320 elems
T15 = 15 * D15         # 982680 elems


def _copy_plan(pos, max_pos):
    pos = (np.asarray(pos).astype(np.int64) % MAX_SIZE).ravel()
    next_pos = int(max_pos) + 1
    if next_pos > MAX_SIZE:
        out_rows = MAX_SIZE
        split = next_pos % MAX_SIZE
        order = (np.arange(MAX_SIZE, dtype=np.int64) + split) % MAX_SIZE
    else:
        out_rows = next_pos
        order = np.arange(next_pos, dtype=np.int64)
    newpos = np.full(MAX_SIZE, -1, dtype=np.int64)
    newpos[pos] = np.arange(pos.shape[0], dtype=np.int64)
    sel = newpos[order]
    is_new = sel >= 0
    src_row = np.where(is_new, sel, order)
    runs = []
    j = 0
    while j < out_rows:
        s = j
        while (
            j + 1 < out_rows
            and is_new[j + 1] == is_new[s]
            and src_row[j + 1] == src_row[s] + (j + 1 - s)
        ):
            j += 1
        runs.append((s, "new" if is_new[s] else "cache", int(src_row[s]), j - s + 1))
        j += 1
    return out_rows, runs


def _chunk_run(L):
    """Chunk a run of L elems (L % 16 == 0, runs are row-granular) into
    instruction sizes: 16-desc big chunks (T16) with an even number of
    15-desc chunks (T15) mixed in for the engine-15 bias, plus a 16-desc
    tail (tail/16 <= 65535 guaranteed since tail < T16)."""
    out = []
    n15 = min((L // T15) & ~1, 4)  # up to two pairs per run
    rem = L - n15 * T15
    n16 = rem // T16
    tail = rem - n16 * T16
    # interleave: spread the 15-desc pair inside the 16-desc stream
    mid = n16 // 2
    out += [T16] * mid
    out += [T15] * n15
    out += [T16] * (n16 - mid)
    if tail > 0:
        out.append(tail)
    return out


def _build(out_rows, runs):
    import concourse.bass as bass
    import concourse.mybir as mybir

    nc = bass.Bass(enable_partition_id=False, dynamic_dma_scratch_size=2048)
    f16 = mybir.dt.float16
    kc = nc.declare_dram_parameter("kc", [MAX_SIZE, HD], f16, isOutput=False)
    vc = nc.declare_dram_parameter("vc", [MAX_SIZE, HD], f16, isOutput=False)
    kn = nc.declare_dram_parameter("kn", [S_NEW, HD], f16, isOutput=False)
    vn = nc.declare_dram_parameter("vn", [S_NEW, HD], f16, isOutput=False)
    ko = nc.declare_dram_parameter("ko", [out_rows, HD], f16, isOutput=True)
    vo = nc.declare_dram_parameter("vo", [out_rows, HD], f16, isOutput=True)

    def flat(t):
        return t[:, :].rearrange("a b -> (a b)")

    kc_f, kn_f, ko_f = flat(kc), flat(kn), flat(ko)
    vc_f, vn_f, vo_f = flat(vc), flat(vn), flat(vo)

    def chunks_for(runs, new_f, cache_f):
        ch = []  # (dst_off, src_flat, src_off, T)
        for dst, src, row, n in runs:
            sv = new_f if src == "new" else cache_f
            so, do = row * HD, dst * HD
            done = 0
            for t in _chunk_run(n * HD):
                ch.append((do + done, sv, so + done, t))
                done += t
        # tiny chunks first (wake the engines), then the rest in given order
        ch.sort(key=lambda c: (c[3] > 2 * HD,))
        return ch

    k_chunks = chunks_for(runs, kn_f, kc_f)
    v_chunks = chunks_for(runs, vn_f, vc_f)

    with (
        nc.Block(no_gpsimd_drain=True) as block,
        nc.semaphore("k_sem") as k_sem,
        nc.semaphore("v_sem") as v_sem,
    ):

        @block.sync
        def _(sync):
            for do, sv, so, t in k_chunks:
                sync.dma_start(
                    out=ko_f[do : do + t], in_=sv[so : so + t]
                ).then_inc(k_sem, 16)
            sync.wait_ge(k_sem, 16 * len(k_chunks))

        @block.scalar
        def _(scalar):
            for do, sv, so, t in v_chunks:
                scalar.dma_start(
                    out=vo_f[do : do + t], in_=sv[so : so + t]
                ).then_inc(v_sem, 16)
            scalar.wait_ge(v_sem, 16 * len(v_chunks))

    return nc


def _run(k, v, k_cache, v_cache, pos, max_pos, trace=False):
    from concourse.bass_utils import run_bass_kernel_spmd

    k = np.asarray(k)
    v = np.asarray(v)
    k_cache = np.asarray(k_cache)
    v_cache = np.asarray(v_cache)

    out_rows, runs = _copy_plan(pos, max_pos)
    nc = _build(out_rows, runs)

    in_maps = [
        {
            "kc": k_cache[b].reshape(MAX_SIZE, HD),
            "vc": v_cache[b].reshape(MAX_SIZE, HD),
            "kn": k[b].reshape(S_NEW, HD),
            "vn": v[b].reshape(S_NEW, HD),
        }
        for b in range(N_CORES)
    ]
    res = run_bass_kernel_spmd(nc, in_maps, list(range(N_CORES)), trace=trace)
    k_out = np.stack([r["ko"] for r in res.results]).reshape(B, out_rows, H, D)
    v_out = np.stack([r["vo"] for r in res.results]).reshape(B, out_rows, H, D)
    return (k_out, v_out), res


def kernel(k, v, k_cache, v_cache, pos, max_pos):
    (k_out, v_out), _ = _run(k, v, k_cache, v_cache, pos, max_pos)
    return k_out, v_out


# revision 5
# speedup vs baseline: 1.0031x; 1.0031x over previous
"""KV-cache ring-buffer update + rolling re-linearization on 8 trn2 cores.

Problem semantics (nn_KVCache): scatter k/v into the ring buffer at pos,
then roll to logical order. For the given pos/max_pos this reduces to
contiguous row copies per batch:
  out[:, 0:7168]    = cache[:, 512:7680]
  out[:, 7168:8192] = new[:, 0:1024]
Sharding: pure batch-parallel (B=8 -> 1 batch per core), no communication.
Each core runs DRAM->DRAM HWDGE DMA copies on both hardware DGE queues.

DMA schedule per core (established empirically by tracing):
- HWDGE assigns ceil(c/16) descriptors per SDMA engine, filling engines from
  slot 0: a 15-desc instruction puts nothing on engine 15 (the slow one).
- 64-KiB descriptors (the HW max: 16-bit byte-size ISA field).
- ~34 instructions per queue (dispatch costs ~850 ns each on the issuing
  engine, so instruction count is kept moderate).
- Engine 15 carries ~0.75 of the other engines' bytes: it runs 16.3-17.5
  vs 20.5-21 B/ns in about half of runs, and its slow-day rate varies, so
  the bias leaves margin rather than balancing at the average (validated:
  0.75 beat 0.81 across 6 clean samples; engine 15 never gates).
"""

import numpy as np

B, S_NEW, H, D = 8, 1024, 16, 128
MAX_SIZE = 8192
HD = H * D
N_CORES = 8

D16 = 32768            # 16-desc chunk descriptor size (elems), 65536 B (HW max)
D15 = 32760            # 15-desc chunk descriptor size (elems), 65520 B
T16 = 16 * D16         # 1048320 elems
T15 = 15 * D15         # 982680 elems


def _copy_plan(pos, max_pos):
    pos = (np.asarray(pos).astype(np.int64) % MAX_SIZE).ravel()
    next_pos = int(max_pos) + 1
    if next_pos > MAX_SIZE:
        out_rows = MAX_SIZE
        split = next_pos % MAX_SIZE
        order = (np.arange(MAX_SIZE, dtype=np.int64) + split) % MAX_SIZE
    else:
        out_rows = next_pos
        order = np.arange(next_pos, dtype=np.int64)
    newpos = np.full(MAX_SIZE, -1, dtype=np.int64)
    newpos[pos] = np.arange(pos.shape[0], dtype=np.int64)
    sel = newpos[order]
    is_new = sel >= 0
    src_row = np.where(is_new, sel, order)
    runs = []
    j = 0
    while j < out_rows:
        s = j
        while (
            j + 1 < out_rows
            and is_new[j + 1] == is_new[s]
            and src_row[j + 1] == src_row[s] + (j + 1 - s)
        ):
            j += 1
        runs.append((s, "new" if is_new[s] else "cache", int(src_row[s]), j - s + 1))
        j += 1
    return out_rows, runs


def _chunk_run(L):
    """Chunk a run of L elems (L % 16 == 0, runs are row-granular) into
    instruction sizes: 16-desc big chunks (T16) with an even number of
    15-desc chunks (T15) mixed in for the engine-15 bias, plus a 16-desc
    tail (tail/16 <= 65535 guaranteed since tail < T16)."""
    out = []
    n15 = min((L // T15) & ~1, 4)  # up to two pairs per run
    rem = L - n15 * T15
    n16 = rem // T16
    tail = rem - n16 * T16
    # interleave: spread the 15-desc pair inside the 16-desc stream
    mid = n16 // 2
    out += [T16] * mid
    out += [T15] * n15
    out += [T16] * (n16 - mid)
    if tail > 0:
        out.append(tail)
    return out


def _build(out_rows, runs):
    import concourse.bass as bass
    import concourse.mybir as mybir

    nc = bass.Bass(enable_partition_id=False, dynamic_dma_scratch_size=2048)
    f16 = mybir.dt.float16
    kc = nc.declare_dram_parameter("kc", [MAX_SIZE, HD], f16, isOutput=False)
    vc = nc.declare_dram_parameter("vc", [MAX_SIZE, HD], f16, isOutput=False)
    kn = nc.declare_dram_parameter("kn", [S_NEW, HD], f16, isOutput=False)
    vn = nc.declare_dram_parameter("vn", [S_NEW, HD], f16, isOutput=False)
    ko = nc.declare_dram_parameter("ko", [out_rows, HD], f16, isOutput=True)
    vo = nc.declare_dram_parameter("vo", [out_rows, HD], f16, isOutput=True)

    def flat(t):
        return t[:, :].rearrange("a b -> (a b)")

    kc_f, kn_f, ko_f = flat(kc), flat(kn), flat(ko)
    vc_f, vn_f, vo_f = flat(vc), flat(vn), flat(vo)

    def chunks_for(runs, new_f, cache_f):
        ch = []  # (dst_off, src_flat, src_off, T)
        for dst, src, row, n in runs:
            sv = new_f if src == "new" else cache_f
            so, do = row * HD, dst * HD
            done = 0
            for t in _chunk_run(n * HD):
                ch.append((do + done, sv, so + done, t))
                done += t
        # tiny chunks first (wake the engines), then the rest in given order
        ch.sort(key=lambda c: (c[3] > 2 * HD,))
        return ch

    k_chunks = chunks_for(runs, kn_f, kc_f)
    v_chunks = chunks_for(runs, vn_f, vc_f)

    with (
        nc.Block(no_gpsimd_drain=True) as block,
        nc.semaphore("k_sem") as k_sem,
        nc.semaphore("v_sem") as v_sem,
    ):

        @block.sync
        def _(sync):
            for do, sv, so, t in k_chunks:
                sync.dma_start(
                    out=ko_f[do : do + t], in_=sv[so : so + t]
                ).then_inc(k_sem, 16)
            sync.wait_ge(k_sem, 16 * len(k_chunks))

        @block.scalar
        def _(scalar):
            for do, sv, so, t in v_chunks:
                scalar.dma_start(
                    out=vo_f[do : do + t], in_=sv[so : so + t]
                ).then_inc(v_sem, 16)
            scalar.wait_ge(v_sem, 16 * len(v_chunks))

    return nc


def _run(k, v, k_cache, v_cache, pos, max_pos, trace=False):
    from concourse.bass_utils import run_bass_kernel_spmd

    k = np.asarray(k)
    v = np.asarray(v)
    k_cache = np.asarray(k_cache)
    v_cache = np.asarray(v_cache)

    out_rows, runs = _copy_plan(pos, max_pos)
    nc = _build(out_rows, runs)

    in_maps = [
        {
            "kc": k_cache[b].reshape(MAX_SIZE, HD),
            "vc": v_cache[b].reshape(MAX_SIZE, HD),
            "kn": k[b].reshape(S_NEW, HD),
            "vn": v[b].reshape(S_NEW, HD),
        }
        for b in range(N_CORES)
    ]
    res = run_bass_kernel_spmd(nc, in_maps, list(range(N_CORES)), trace=trace)
    k_out = np.stack([r["ko"] for r in res.results]).reshape(B, out_rows, H, D)
    v_out = np.stack([r["vo"] for r in res.results]).reshape(B, out_rows, H, D)
    return (k_out, v_out), res


def kernel(k, v, k_cache, v_cache, pos, max_pos):
    (k_out, v_out), _ = _run(k, v, k_cache, v_cache, pos, max_pos)
    return k_out, v_out
